# revision 25
# baseline (speedup 1.0000x reference)
"""Segment mean-pool (LocalPooling1D) Trainium2 Bass kernel.

x [32, 8192, 256] f32, x_pos [32, 65] sorted int32 boundaries -> y [32, 64, 256].
y[b, j] = mean(x[b, x_pos[b,j]:x_pos[b,j+1]]), empty segments -> 0.

Strategy: data-parallel over batch, 4 rows per core on 8 cores. The kernel is
HBM-bound (33.55 MB of x per core; the 16-engine DMA pool peaks at ~410 GB/s
per core), so everything is arranged to keep the DMA engines saturated
end-to-end:

- Token-interleaved x layout: token t = 512*chunk + 4*p + k lives in
  partition p, slot k, so each partition line of a chunk tile is 4 KB of
  contiguous DRAM, split into descriptors via max_dma_last_dim. Each of the
  two hw DGE queues (sync + scalar engines, alternating chunks) feeds its
  own half of the engine pool at ~1 descriptor / 5 ns, so both queues lead
  with x chunks from their first instruction.
- Demand shaping against the HAM power governor (which clamps 1-2 random
  cores to ~300 GB/s from ~35 us in, insensitive to demand once clamped):
  the first 16 MB go out at 4 KB descriptors (~410 GB/s) inside the
  governor's grace window, the rest alternates 1 KB / 768 B (~358 GB/s, the
  measured optimum). The kernel's final chunk lands as per-slice DMAs so
  tail matmuls overlap the last arrivals.
- The 0/1 segment indicator ind[p, m, j] = (pos[j] <= t < pos[j+1]) is built
  per row in half-row fused DVE ops from stride-0 broadcast views
  (scalar_tensor_tensor compare, then a shifted subtract); pos is replicated
  across partitions with a tiny PE matmul (ones.T @ posf) so no slow-queue
  dependency sits in front of the x stream. ind_bufs=2 defers rows 2-3's
  builds past the bank phase (provably in time: the PE runs rows in order).
- Segment sums accumulate as psum += ind.T @ x_slice in single-pass fp32r
  (weights are exactly 0/1; only x's mantissa rounds -- rel err ~1e-4).
  psum_bufs=4 gives each row its own PSUM bank so scales never block the
  matmul stream. y = psum * 1/max(count, 1); mid-stream rows write back on
  the idle gpsimd queue, the final row on sync's drained hw queue.
"""

import os
import sys

import numpy as np

sys.path.insert(0, "/opt/trn_rl_repo")

import concourse.bacc as bacc
import concourse.bass as bass
import concourse.tile as tile
from concourse import mybir
from concourse.bass_utils import run_bass_kernel_spmd

dt = mybir.dt
Alu = mybir.AluOpType

# Problem constants (hardcoded per harness contract).
B, T, C, P = 32, 8192, 256, 65
NSEG = P - 1
NCORES = 8
R = B // NCORES          # batch rows per core
TOK = 128                # partitions (matmul contraction dim)
K = 4                    # tokens per partition line (4 KB contiguous)
CHTOK = TOK * K          # 512 tokens per chunk
CH = T // CHTOK          # 16 chunks per row

CFG = {
    "x_bufs": int(os.environ.get("KB_XBUFS", "20")),
    # 2 ind bufs also serve as an energy deferral: row r+2's indicator
    # build waits for row r's matmuls (always in time, the PE runs rows in
    # order), keeping wide DVE work out of the pre-governor bank phase.
    "ind_bufs": int(os.environ.get("KB_INDBUFS", "2")),
    "s_bufs": int(os.environ.get("KB_SBUFS", "2")),
    "psum_bufs": int(os.environ.get("KB_PSUMBUFS", "4")),
    # DMA descriptor size in f32 elems: 1024 -> 4KB (per-core demand ~410
    # GB/s), 512 -> 2KB (~384), 256 -> 1KB (~363). Smaller descriptors cap
    # per-core HBM demand nearer the 8-core fair share, reducing HAM
    # duty-cycle throttling of unlucky cores.
    "desc_elems": int(os.environ.get("KB_DESC", "256")),
    # First N global chunks use full 4KB descriptors (the HAM power governor
    # only reacts ~25 us in, so the ramp phase can run at the ~410 GB/s peak
    # before settling to the smooth rate).
    "fast_chunks": int(os.environ.get("KB_FASTCH", "32")),
    # Post-ramp, alternate 1KB / 768B descriptors: queues dispatch ~1
    # descriptor / 5 ns, so this lands ~358 GB/s demand (1KB everywhere
    # still reaches ~408 -- measured, not the naive 363).
    "mix192": os.environ.get("KB_MIX192", "1") == "1",
}


def build_program(cfg=CFG):
    nc = bacc.Bacc("TRN2", target_bir_lowering=False, debug=False)

    x_d = nc.dram_tensor("x", [R, T, C], dt.float32, kind="ExternalInput")
    pos_d = nc.dram_tensor("x_pos", [R, P], dt.int32, kind="ExternalInput")
    y_d = nc.dram_tensor("y", [R, NSEG, C], dt.float32, kind="ExternalOutput")

    with tile.TileContext(nc) as tc:
        with (
            tc.tile_pool(name="const", bufs=1) as constp,
            tc.tile_pool(name="xp", bufs=cfg["x_bufs"]) as xp,
            tc.tile_pool(name="sp", bufs=cfg["s_bufs"]) as sp,
            tc.tile_pool(name="indp", bufs=cfg["ind_bufs"]) as indp,
            tc.tile_pool(name="smallp", bufs=1) as smallp,
            tc.tile_pool(name="outp", bufs=2) as outp,
            tc.tile_pool(name="psp", bufs=cfg["psum_bufs"], space="PSUM") as psp,
            tc.tile_pool(name="pbp", bufs=1, space="PSUM") as pbp,
        ):
            # 512*ch + k over the (chunk, slot) axes; identical per partition.
            # All values < 2^13, exact in f32.
            tio2 = constp.tile([TOK, CH * K], dt.float32)
            nc.gpsimd.iota(tio2[:], pattern=[[CHTOK, CH], [1, K]], base=0,
                           channel_multiplier=0,
                           allow_small_or_imprecise_dtypes=True)
            # 4*p as a per-partition scalar.
            p4 = constp.tile([TOK, 1], dt.float32)
            nc.gpsimd.iota(p4[:], pattern=[[1, 1]], base=0,
                           channel_multiplier=K,
                           allow_small_or_imprecise_dtypes=True)
            ones_row = constp.tile([1, TOK], dt.float32)
            nc.vector.memset(ones_row[:], 1.0)

            # Each hw queue feeds its own half of the DMA engine pool, so
            # BOTH queues must lead with x. Only the tiny pos_i load (4
            # descriptors, ~0.3 us) precedes x on sync; the transposed count
            # loads (512 tiny descriptors = ~2.6 us of queue dispatch) are
            # emitted after row 0's chunk issues -- their data is not needed
            # until the first scale (~35 us in).
            pos_i = smallp.tile([1, R * P], dt.int32)
            nc.sync.dma_start(
                pos_i[0:1, :].rearrange("one (r p) -> one r p", r=R),
                pos_d[:, :],
            )
            posf = smallp.tile([1, R * P], dt.float32)
            nc.vector.tensor_copy(posf[:], pos_i[:])
            # Replicate to all 128 partitions via PE: ones.T @ posf (exact).
            pos_b = pbp.tile([TOK, R * P], dt.float32)
            nc.tensor.matmul(pos_b[:], ones_row[:], posf[:], start=True,
                             stop=True)
            pos_lo = smallp.tile([NSEG, R], dt.int32)
            pos_hi = smallp.tile([NSEG, R], dt.int32)

            for r in range(R):
                # S[p, m, j] = (pos[j] - 4p <= 512*(m//4) + m%4) with
                # m = (ch, k) collapsed (DVE ops allow at most 2 free dims);
                # fused DVE ops over two stride-0 broadcast views, built in
                # halves so the first matmuls can start ~2.7 us earlier and
                # row boundaries react faster.
                M = CH * K
                S_all = sp.tile([TOK, M, P], dt.float32, tag="sall")
                ind_all = indp.tile([TOK, M, NSEG], dt.float32r, tag="ind")
                for h0, h1 in ((0, M // 2), (M // 2, M)):
                    nc.vector.scalar_tensor_tensor(
                        S_all[:, h0:h1, :],
                        pos_b[:, r * P : (r + 1) * P][:, None, :]
                            .broadcast_to((TOK, h1 - h0, P)),
                        p4[:],
                        tio2[:, h0:h1, None].broadcast_to((TOK, h1 - h0, P)),
                        op0=Alu.subtract,
                        op1=Alu.is_le,
                    )
                    # ind[p, m, j] = S[p, m, j] - S[p, m, j+1]
                    nc.vector.tensor_tensor(
                        ind_all[:, h0:h1, :], S_all[:, h0:h1, 0:NSEG],
                        S_all[:, h0:h1, 1:P], op=Alu.subtract,
                    )

                # fp32r: single-pass fp32 matmul (1 cycle/row at N=256 vs 4
                # for two-pass fp32; col-group packing is not supported with
                # fp32r, but at this rate the PE is only ~30% busy anyway).
                # Weights are exactly 0/1 and PSUM accumulates in fp32, so
                # the only precision loss is the moving x operand's mantissa
                # rounding -- far inside the tolerance.
                ps = psp.tile([NSEG, C], dt.float32)
                xr = x_d[r].rearrange("(ch p k) c -> ch p (k c)", p=TOK, k=K)
                for ch in range(CH):
                    xt = xp.tile([TOK, K * C], dt.float32r)
                    g = r * CH + ch
                    eng = nc.sync if g % 2 == 0 else nc.scalar
                    de = 1024 if g < cfg["fast_chunks"] else cfg["desc_elems"]
                    if de != 1024 and cfg.get("mix192"):
                        cyc = cfg.get("mix_cycle", 2)
                        if (g // 2) % cyc:
                            de = 192
                    last = r == R - 1 and ch == CH - 1
                    if last:
                        # Final chunk of the kernel: land it as K per-slice
                        # DMAs (alternating queues) so each tail matmul can
                        # start as soon as its slice arrives instead of
                        # waiting for the whole 4KB-line chunk.
                        xrk = x_d[r].rearrange("(ch p k) c -> ch k p c",
                                               p=TOK, k=K)
                        for k in range(K):
                            e2 = nc.sync if k % 2 == 0 else nc.scalar
                            e2.dma_start(
                                xt[:, k * C : (k + 1) * C],
                                xrk[ch, k].bitcast(dt.float32r),
                                max_dma_last_dim=cfg["desc_elems"],
                            )
                    else:
                        eng.dma_start(xt[:], xr[ch].bitcast(dt.float32r),
                                      max_dma_last_dim=de)
                    for k in range(K):
                        nc.tensor.matmul(
                            ps[:], ind_all[:, ch * K + k, :],
                            xt[:, k * C : (k + 1) * C],
                            start=(ch == 0 and k == 0),
                            stop=(ch == CH - 1 and k == K - 1),
                        )

                if r == 0:
                    # counts -> 1/max(cnt, 1), partition-major [NSEG, R];
                    # issued here so the transposed loads queue behind row
                    # 0's x chunks instead of ahead of them.
                    nc.scalar.dma_start(
                        pos_lo[:], pos_d[:, 0:NSEG].rearrange("r p -> p r"))
                    nc.scalar.dma_start(
                        pos_hi[:], pos_d[:, 1:P].rearrange("r p -> p r"))
                    cnt_f = smallp.tile([NSEG, R], dt.float32)
                    nc.vector.tensor_tensor(cnt_f[:], pos_hi[:], pos_lo[:],
                                            op=Alu.subtract)
                    cntc = smallp.tile([NSEG, R], dt.float32)
                    nc.vector.tensor_scalar(cntc[:], cnt_f[:], 1.0, None,
                                            op0=Alu.max)
                    recip = smallp.tile([NSEG, R], dt.float32)
                    nc.vector.reciprocal(recip[:], cntc[:])

                # y = psum * 1/max(cnt, 1)
                out_t = outp.tile([NSEG, C], dt.float32, tag="out")
                nc.vector.tensor_scalar(
                    out_t[:], ps[:], recip[:, r : r + 1], None, op0=Alu.mult
                )
                # Mid-stream y writes ride the gpsimd SWDGE queue (latency
                # hidden); the final row's write goes out on sync's hw queue,
                # which has no x issues left by then -- its ~1.5 us lower
                # latency is pure tail savings.
                yeng = nc.sync if r == R - 1 else nc.gpsimd
                yeng.dma_start(y_d[r], out_t[:])

    nc.compile()
    return nc


_PROGRAM = None


def _get_program():
    global _PROGRAM
    if _PROGRAM is None:
        _PROGRAM = build_program()
    return _PROGRAM


def kernel(x, x_pos):
    x = np.ascontiguousarray(x, dtype=np.float32)
    x_pos = np.ascontiguousarray(x_pos, dtype=np.int32)
    nc = _get_program()
    in_maps = [
        {"x": x[c * R : (c + 1) * R], "x_pos": x_pos[c * R : (c + 1) * R]}
        for c in range(NCORES)
    ]
    res = run_bass_kernel_spmd(nc, in_maps, list(range(NCORES)))
    y = np.concatenate([res.results[c]["y"] for c in range(NCORES)], axis=0)
    return y.astype(np.float32)
